# revision 15
# baseline (speedup 1.0000x reference)
"""Tensor-parallel LlamaAttention (GQA + RoPE + causal) for 8 trn2 NeuronCores.

v5 design (on top of v4):
  - q/k/v projections run in fp8(e4m3) DoubleRow perf mode: host quantizes
    x / wq / wk / wv per-tensor to +-240; contraction tiles are [128, 2, F]
    h-block-pairs so each matmul contracts 256 at 0.5 cyc/row. Dequant
    scales fold into the exp scale (q,k) and the vbig rescale (v).
  - o_proj runs in fp8 DoubleRow too: vbig is rescaled so |vbig| <= 230,
    making the normalized attention output (a convex combination) fp8-safe;
    the normalization multiply writes float8e4 directly, the AllToAll moves
    fp8, and the final y copy applies the single dequant constant.
  - Attention is causally trimmed: the two diagonal k-block pairs of each
    q-block use shortened moving ranges (512+384 and 256+128 columns) with
    a single shared [128,128] triangular mask, cutting scores/exp/PV work
    on the diagonal by ~37%.
  - Softmax denominator uses a single [128,512] T accumulator (one
    ones-matmul per block instead of two).
  - o_proj chunk order interleaves the (a0,a1) and (a2,a3) pair groups so
    the last AllToAll is hidden behind (a0,a1) compute.
"""

import math
import sys

import numpy as np

sys.path.insert(0, "/opt/trn_rl_repo")

import ml_dtypes  # noqa: E402

from concourse import bacc, bass_isa, mybir, tile  # noqa: E402
from concourse.bass_utils import run_bass_kernel_spmd  # noqa: E402

F32 = mybir.dt.float32
BF16 = mybir.dt.bfloat16
F8 = mybir.dt.float8e4
DR = mybir.MatmulPerfMode.DoubleRow
NCORES = 8
P = 128  # partitions / head dim
QB = 512  # q-block (PSUM free dim)
KB = 128  # k-block (contraction tile)

import os
FP8_QKV = os.environ.get("K_FP8_QKV", "0") == "1"
FP8_O = os.environ.get("K_FP8_O", "0") == "1"
VCLIP = 230.0  # |vbig| bound; keeps attn output < 240 fp8 max

_CACHE = {}


def build_program(B, S, H, NH, NKV, escale, vscale_dev, yscale):
    nc = bacc.Bacc("TRN2", num_devices=NCORES)

    BT = B * S
    NHC = NH // NCORES  # q heads per core
    assert NKV // NCORES == 1
    assert NH * P == H
    DQ = NHC * P  # per-core attn width
    HB = H // P  # h blocks
    HBP = HB // 2  # h block pairs (256-contraction steps)
    TB = BT // QB  # token super-blocks
    QBB = S // QB  # q blocks per batch
    GQA = min(2, QBB)  # q-blocks per AllToAll
    NQA = (QBB + GQA - 1) // GQA  # AllToAlls per batch
    NA = B * NQA  # total AllToAlls
    SCA = GQA * QB // NCORES  # tokens per (core, a2a)
    CPQ = QB // SCA  # a2a chunks per q-block
    OCB = H // QB  # o_proj output column chunks
    DPAIRS = (NH * P) // 256  # o_proj contraction double-blocks
    DPG = min(4, DPAIRS)  # dpairs per a2a-out load tile

    XDT = F8 if FP8_QKV else BF16
    ADT = F8 if FP8_O else BF16
    xdw = 1 if FP8_QKV else 2  # bytes
    adw = 1 if FP8_O else 2

    xT = nc.declare_dram_parameter("xT", [H, BT], XDT, isOutput=False)
    wq_c = nc.declare_dram_parameter("wq_c", [H, DQ], XDT, isOutput=False)
    wk_c = nc.declare_dram_parameter("wk_c", [H, P], XDT, isOutput=False)
    wv_c = nc.declare_dram_parameter("wv_c", [H, P], XDT, isOutput=False)
    wo_full = nc.declare_dram_parameter("wo_full", [NH * P, H], ADT, isOutput=False)
    cos_t = nc.declare_dram_parameter("cos_t", [P, BT], F32, isOutput=False)
    sinx_t = nc.declare_dram_parameter("sinx_t", [P, BT], F32, isOutput=False)
    tri_t = nc.declare_dram_parameter("tri_t", [P, P], BF16, isOutput=False)
    ones_t = nc.declare_dram_parameter("ones_t", [P, P], BF16, isOutput=False)
    y_c = nc.declare_dram_parameter("y_c", [NA * SCA, H], F32, isOutput=True)

    def mm_pair(out, lhsT, rhs, start, stop, fp8):
        """One 256-contraction step: DoubleRow if fp8 else two bf16 matmuls.
        lhsT: [128, 2, M]; rhs: [128, 2, N]; out: [M, N]."""
        if fp8:
            nc.tensor.matmul(out, lhsT, rhs, start=start, stop=stop, perf_mode=DR)
        else:
            nc.tensor.matmul(out, lhsT[:, 0], rhs[:, 0], start=start, stop=False)
            nc.tensor.matmul(out, lhsT[:, 1], rhs[:, 1], start=False, stop=stop)

    with tile.TileContext(nc) as tc:
        with (
            tc.tile_pool(name="dram", bufs=1, space="DRAM") as dram,
            tc.tile_pool(name="const", bufs=1) as constp,
            tc.tile_pool(name="persist", bufs=1) as persist,
        ):
            a2a_in = [
                dram.tile([NCORES * DQ, SCA], ADT, tag=f"a2ain{a}", name=f"a2ain{a}")
                for a in range(NA)
            ]
            a2a_out = [
                dram.tile([NCORES * DQ, SCA], ADT, tag=f"a2aout{a}", name=f"a2aout{a}")
                for a in range(NA)
            ]

            ones_sb = constp.tile([P, P], BF16, tag="ones")
            nc.sync.dma_start(out=ones_sb, in_=ones_t[:, :])
            tri_sb = constp.tile([P, P], BF16, tag="tri")
            nc.sync.dma_start(out=tri_sb, in_=tri_t[:, :])

            qT = [
                persist.tile([P, BT], BF16, tag=f"qT{i}", name=f"qT{i}")
                for i in range(NHC)
            ]
            kT = persist.tile([P, BT], BF16, tag="kT")
            vbig = persist.tile([P, BT], BF16, tag="vbig")

            # ---------------- phase 1: q/k/v projections + rope
            with (
                tc.tile_pool(name="xin", bufs=(3 * HBP) // 2) as xin_p,
                tc.tile_pool(name="wqkv", bufs=1) as w_p,
                tc.tile_pool(name="tabs", bufs=2) as tab_p,
                tc.tile_pool(name="ropetmp", bufs=4) as rt_p,
                tc.tile_pool(name="psq", bufs=1, space="PSUM") as psq_p,
                tc.tile_pool(name="psk", bufs=2, space="PSUM") as psk_p,
                tc.tile_pool(name="psv", bufs=2, space="PSUM") as psv_p,
            ):
                wk_big = w_p.tile([P, HBP, 2, P], XDT, tag="wkb")
                wv_big = w_p.tile([P, HBP, 2, P], XDT, tag="wvb")
                wq_sb = [
                    w_p.tile([P, 2, DQ], XDT, tag=f"wq{i}", name=f"wq{i}")
                    for i in range(HBP)
                ]
                cos0 = tab_p.tile([P, QB], F32, tag="cos")
                sinx0 = tab_p.tile([P, QB], F32, tag="sinx")
                # x/wq stream on the sync queue so the first q chain starts
                # immediately; bulk wk/wv + tables go on the idle scalar queue
                xts0 = []
                for hp in range(HBP):
                    xi = xin_p.tile([P, 2, QB], XDT, tag="xin", name=f"xin0_{hp}")
                    nc.sync.dma_start(
                        out=xi,
                        in_=xT[hp * 256 : (hp + 1) * 256, 0:QB].rearrange(
                            "(j p) t -> p j t", j=2
                        ),
                    )
                    xts0.append(xi)
                    nc.sync.dma_start(
                        out=wq_sb[hp],
                        in_=wq_c[hp * 256 : (hp + 1) * 256, :].rearrange(
                            "(j p) d -> p j d", j=2
                        ),
                    )
                nc.scalar.dma_start(out=cos0, in_=cos_t[:, 0:QB])
                nc.scalar.dma_start(out=sinx0, in_=sinx_t[:, 0:QB])
                nc.scalar.dma_start(
                    out=wk_big,
                    in_=wk_c[:, :].rearrange("(g j p) d -> p g j d", g=HBP, j=2),
                )
                nc.scalar.dma_start(
                    out=wv_big,
                    in_=wv_c[:, :].rearrange("(g j p) d -> p g j d", g=HBP, j=2),
                )

                def rope(dst, ps, cos_sb, sinx_sb):
                    """dst = ps*cos + shift64(ps)*sinx (all [128,QB])"""
                    t1 = rt_p.tile([P, QB], F32, tag="ropet1")
                    t2 = rt_p.tile([P, QB], F32, tag="ropet2")
                    nc.vector.tensor_tensor(t1, ps, cos_sb, mybir.AluOpType.mult)
                    h = P // 2
                    nc.vector.tensor_tensor(
                        t2[0:h], ps[h:P], sinx_sb[0:h], mybir.AluOpType.mult
                    )
                    nc.vector.tensor_tensor(
                        t2[h:P], ps[0:h], sinx_sb[h:P], mybir.AluOpType.mult
                    )
                    nc.vector.tensor_tensor(dst, t1, t2, mybir.AluOpType.add)

                dqgroups = [
                    tuple(range(g, min(g + 2, NHC))) for g in range(0, NHC, 2)
                ]

                for tb in range(TB):
                    t0 = tb * QB
                    if tb == 0:
                        cos_sb, sinx_sb, xts = cos0, sinx0, xts0
                    else:
                        cos_sb = tab_p.tile([P, QB], F32, tag="cos")
                        sinx_sb = tab_p.tile([P, QB], F32, tag="sinx")
                        nc.sync.dma_start(out=cos_sb, in_=cos_t[:, t0 : t0 + QB])
                        nc.sync.dma_start(out=sinx_sb, in_=sinx_t[:, t0 : t0 + QB])
                        xts = []
                        for hp in range(HBP):
                            xi = xin_p.tile([P, 2, QB], XDT, tag="xin")
                            nc.sync.dma_start(
                                out=xi,
                                in_=xT[
                                    hp * 256 : (hp + 1) * 256, t0 : t0 + QB
                                ].rearrange("(j p) t -> p j t", j=2),
                            )
                            xts.append(xi)

                    v_ps = psv_p.tile([P, QB], F32, tag="vps")
                    vchains = list(range(QB // P))
                    groups = [("q", grp) for grp in dqgroups] + [("k", None)]
                    while len(groups) < len(vchains):
                        groups.append(("", None))
                    for gi, (kind, grp) in enumerate(groups):
                        vi = vchains[gi] if gi < len(vchains) else None
                        q_ps = {}
                        if kind == "q":
                            q_ps = {
                                dq: psq_p.tile(
                                    [P, QB], F32, tag=f"qps{dq}", name=f"qps{dq}",
                                    bufs=(2 if NHC == 1 else 1),
                                )
                                for dq in grp
                            }
                        elif kind == "k":
                            k_ps = psk_p.tile([P, QB], F32, tag="kps")
                        for hp in range(HBP):
                            st, sp = (hp == 0), (hp == HBP - 1)
                            if kind == "q":
                                for dq in grp:
                                    mm_pair(
                                        q_ps[dq],
                                        wq_sb[hp][:, :, dq * P : (dq + 1) * P],
                                        xts[hp],
                                        st, sp, FP8_QKV,
                                    )
                            elif kind == "k":
                                mm_pair(
                                    k_ps, wk_big[:, hp], xts[hp], st, sp, FP8_QKV
                                )
                            if vi is not None:
                                mm_pair(
                                    v_ps[:, vi * P : (vi + 1) * P],
                                    xts[hp][:, :, vi * P : (vi + 1) * P],
                                    wv_big[:, hp],
                                    st, sp, FP8_QKV,
                                )
                        if kind == "q":
                            for dq in grp:
                                rope(qT[dq][:, t0 : t0 + QB], q_ps[dq], cos_sb, sinx_sb)
                        elif kind == "k":
                            rope(kT[:, t0 : t0 + QB], k_ps, cos_sb, sinx_sb)
                    nc.vector.tensor_scalar_mul(
                        vbig[:, t0 : t0 + QB], v_ps, float(vscale_dev)
                    )

            # ---------------- phases 2+3 share the wo + a2a-out pools
            with (
                tc.tile_pool(name="wo", bufs=3 * DPAIRS) as wo_p,
                tc.tile_pool(name="aT", bufs=1) as at_p,
            ):
                at_sb = []  # [a][g] tile [P, DPG, 2, SCA]

                def emit_at_load(a):
                    row = []
                    for g in range(DPAIRS // DPG):
                        t = at_p.tile(
                            [P, DPG, 2, SCA], ADT, tag=f"aT{a}_{g}", name=f"aT{a}_{g}"
                        )
                        src = a2a_out[a][
                            g * DPG * 256 : (g + 1) * DPG * 256, :
                        ].rearrange("(d j p) s -> p d j s", d=DPG, j=2)
                        nc.gpsimd.dma_start(out=t, in_=src)
                        row.append(t)
                    at_sb.append(row)

                wo_sb = {}  # (oc, dp) -> tile [P, 2, QB]
                wo_pending = [(oc, dp) for oc in range(OCB) for dp in range(DPAIRS)]
                wo_ptr = [0]

                def emit_wo(n, cap=None):
                    end = min(wo_ptr[0] + n, cap if cap is not None else len(wo_pending))
                    for idx in range(wo_ptr[0], end):
                        oc, dp = wo_pending[idx]
                        w = wo_p.tile([P, 2, QB], ADT, tag="wo", name=f"wo{oc}_{dp}")
                        nc.sync.dma_start(
                            out=w,
                            in_=wo_full[
                                dp * 256 : (dp + 1) * 256, oc * QB : (oc + 1) * QB
                            ].rearrange("(j p) c -> p j c", j=2),
                        )
                        wo_sb[(oc, dp)] = w
                    wo_ptr[0] = end

                WOPRE = min(3, OCB) * DPAIRS  # tiles prefetched during phase 2

                # ---------------- phase 2: attention + split AllToAlls
                with (
                    tc.tile_pool(name="pP", bufs=3) as p_p,
                    tc.tile_pool(name="pT", bufs=2) as t_p,
                    tc.tile_pool(name="aout", bufs=4) as ao_p,
                    tc.tile_pool(name="rv", bufs=2) as rv_p,
                    tc.tile_pool(name="psS", bufs=2, space="PSUM") as pss_p,
                    tc.tile_pool(name="psO", bufs=2, space="PSUM") as pso_p,
                    tc.tile_pool(name="psL", bufs=1, space="PSUM") as psl_p,
                    tc.tile_pool(name="psR", bufs=1, space="PSUM") as psr_p,
                ):
                    norm_ctr = [0]

                    def emit_norm(st):
                        """deferred normalization tail of a previous iteration:
                        rinv on DVE, PE ones-matmul broadcast, PSUM->SBUF copy
                        alternating between ACT and DVE, DVE multiply."""
                        o_ps, l_ps, aidx, chunk0, h = st
                        rinv = rv_p.tile([1, QB], F32, tag="rinv")
                        nc.vector.reciprocal_approx_fast(out=rinv, in_=l_ps)
                        rinv_b = rv_p.tile([1, QB], BF16, tag="rinvb")
                        nc.vector.tensor_copy(rinv_b, rinv)
                        rb_ps = psr_p.tile([P, QB], F32, tag="rbps")
                        nc.tensor.matmul(
                            rb_ps, ones_sb[0:1, :], rinv_b, start=True, stop=True
                        )
                        rb_sb = rv_p.tile([P, QB], F32, tag="rbsb")
                        if norm_ctr[0] % 2 == 0:
                            nc.scalar.copy(rb_sb, rb_ps)
                        else:
                            nc.vector.tensor_copy(rb_sb, rb_ps)
                        norm_ctr[0] += 1
                        attn_sb = ao_p.tile([P, QB], ADT, tag="attn")
                        nc.vector.tensor_tensor(
                            attn_sb, o_ps, rb_sb, mybir.AluOpType.mult
                        )
                        for cc in range(CPQ):
                            nc.sync.dma_start(
                                out=a2a_in[aidx][
                                    (chunk0 + cc) * DQ + h * P :
                                    (chunk0 + cc) * DQ + (h + 1) * P,
                                    :,
                                ],
                                in_=attn_sb[:, cc * SCA : (cc + 1) * SCA],
                            )

                    def tadd(t_sb, p_ap, lo, first):
                        if first:
                            nc.vector.tensor_copy(t_sb[:, lo:QB], p_ap)
                        else:
                            nc.vector.tensor_tensor(
                                t_sb[:, lo:QB], t_sb[:, lo:QB], p_ap,
                                mybir.AluOpType.add,
                            )

                    prev = None
                    for b in range(B):
                        for qq in range(NQA):
                            aidx = b * NQA + qq
                            for qb in range(qq * GQA, min((qq + 1) * GQA, QBB)):
                                for h in range(NHC):
                                    q0 = b * S + qb * QB
                                    nrect = qb * (QB // KB) // 2  # full pairs
                                    o_ps = pso_p.tile([P, QB], F32, tag="ops")
                                    t_sb = t_p.tile([P, QB], BF16, tag="T")
                                    qTh = qT[h]
                                    for j in range(nrect):
                                        if j == 1 and prev is not None:
                                            emit_norm(prev)
                                            prev = None
                                        k0 = b * S + j * 2 * KB
                                        s_ps = pss_p.tile([P, 2 * QB], F32, tag="sps")
                                        nc.tensor.matmul(
                                            s_ps[:, 0:QB],
                                            kT[:, k0 : k0 + KB],
                                            qTh[:, q0 : q0 + QB],
                                            start=True, stop=True,
                                        )
                                        nc.tensor.matmul(
                                            s_ps[:, QB:],
                                            kT[:, k0 + KB : k0 + 2 * KB],
                                            qTh[:, q0 : q0 + QB],
                                            start=True, stop=True,
                                        )
                                        p_sb = p_p.tile([P, 2 * QB], BF16, tag="P")
                                        nc.scalar.activation(
                                            p_sb, s_ps,
                                            mybir.ActivationFunctionType.Exp,
                                            scale=float(escale),
                                        )
                                        tadd(t_sb, p_sb[:, 0:QB], 0, j == 0)
                                        tadd(t_sb, p_sb[:, QB:], 0, False)
                                        nc.tensor.matmul(
                                            o_ps,
                                            vbig[:, k0 : k0 + P],
                                            p_sb[:, 0:QB],
                                            start=(j == 0), stop=False,
                                        )
                                        nc.tensor.matmul(
                                            o_ps,
                                            vbig[:, k0 + P : k0 + 2 * P],
                                            p_sb[:, QB:],
                                            start=False, stop=False,
                                        )
                                    # ---- diagonal pair jd=0: kb at q0, q0+128
                                    if nrect == 0 and prev is not None:
                                        emit_norm(prev)
                                        prev = None
                                    k0 = q0
                                    s_ps = pss_p.tile([P, 2 * QB], F32, tag="sps")
                                    nc.tensor.matmul(
                                        s_ps[:, 0:QB],
                                        kT[:, k0 : k0 + KB],
                                        qTh[:, q0 : q0 + QB],
                                        start=True, stop=True,
                                    )
                                    nc.tensor.matmul(
                                        s_ps[:, QB : QB + 384],
                                        kT[:, k0 + KB : k0 + 2 * KB],
                                        qTh[:, q0 + 128 : q0 + QB],
                                        start=True, stop=True,
                                    )
                                    p_sb = p_p.tile([P, 2 * QB], BF16, tag="P")
                                    nc.scalar.activation(
                                        p_sb[:, 0 : QB + 384],
                                        s_ps[:, 0 : QB + 384],
                                        mybir.ActivationFunctionType.Exp,
                                        scale=float(escale),
                                    )
                                    nc.vector.tensor_tensor(
                                        p_sb[:, 0:128], p_sb[:, 0:128], tri_sb,
                                        mybir.AluOpType.mult,
                                    )
                                    nc.vector.tensor_tensor(
                                        p_sb[:, QB : QB + 128],
                                        p_sb[:, QB : QB + 128], tri_sb,
                                        mybir.AluOpType.mult,
                                    )
                                    tadd(t_sb, p_sb[:, 0:QB], 0, nrect == 0)
                                    tadd(t_sb, p_sb[:, QB : QB + 384], 128, False)
                                    # split PV so the last writer of every o_ps
                                    # column range carries stop=True
                                    nc.tensor.matmul(
                                        o_ps[:, 0:128],
                                        vbig[:, k0 : k0 + P],
                                        p_sb[:, 0:128],
                                        start=(nrect == 0), stop=True,
                                        skip_group_check=True,
                                    )
                                    nc.tensor.matmul(
                                        o_ps[:, 128:QB],
                                        vbig[:, k0 : k0 + P],
                                        p_sb[:, 128:QB],
                                        start=(nrect == 0), stop=False,
                                        skip_group_check=True,
                                    )
                                    nc.tensor.matmul(
                                        o_ps[:, 128:256],
                                        vbig[:, k0 + P : k0 + 2 * P],
                                        p_sb[:, QB : QB + 128],
                                        start=False, stop=True,
                                        skip_group_check=True,
                                    )
                                    nc.tensor.matmul(
                                        o_ps[:, 256:QB],
                                        vbig[:, k0 + P : k0 + 2 * P],
                                        p_sb[:, QB + 128 : QB + 384],
                                        start=False, stop=False,
                                        skip_group_check=True,
                                    )
                                    # ---- diagonal pair jd=1: kb at q0+256, q0+384
                                    k0 = q0 + 256
                                    s_ps = pss_p.tile([P, 2 * QB], F32, tag="sps")
                                    nc.tensor.matmul(
                                        s_ps[:, 0:256],
                                        kT[:, k0 : k0 + KB],
                                        qTh[:, q0 + 256 : q0 + QB],
                                        start=True, stop=True,
                                    )
                                    nc.tensor.matmul(
                                        s_ps[:, 256:384],
                                        kT[:, k0 + KB : k0 + 2 * KB],
                                        qTh[:, q0 + 384 : q0 + QB],
                                        start=True, stop=True,
                                    )
                                    p_sb = p_p.tile([P, 2 * QB], BF16, tag="P")
                                    nc.scalar.activation(
                                        p_sb[:, 0:384],
                                        s_ps[:, 0:384],
                                        mybir.ActivationFunctionType.Exp,
                                        scale=float(escale),
                                    )
                                    nc.vector.tensor_tensor(
                                        p_sb[:, 0:128], p_sb[:, 0:128], tri_sb,
                                        mybir.AluOpType.mult,
                                    )
                                    nc.vector.tensor_tensor(
                                        p_sb[:, 256:384], p_sb[:, 256:384], tri_sb,
                                        mybir.AluOpType.mult,
                                    )
                                    tadd(t_sb, p_sb[:, 0:256], 256, False)
                                    tadd(t_sb, p_sb[:, 256:384], 384, False)
                                    nc.tensor.matmul(
                                        o_ps[:, 256:384],
                                        vbig[:, k0 : k0 + P],
                                        p_sb[:, 0:128],
                                        start=False, stop=True,
                                        skip_group_check=True,
                                    )
                                    nc.tensor.matmul(
                                        o_ps[:, 384:QB],
                                        vbig[:, k0 : k0 + P],
                                        p_sb[:, 128:256],
                                        start=False, stop=False,
                                        skip_group_check=True,
                                    )
                                    nc.tensor.matmul(
                                        o_ps[:, 384:QB],
                                        vbig[:, k0 + P : k0 + 2 * P],
                                        p_sb[:, 256:384],
                                        start=False, stop=True,
                                        skip_group_check=True,
                                    )
                                    l_ps = psl_p.tile([1, QB], F32, tag="lps")
                                    nc.tensor.matmul(
                                        l_ps, ones_sb[:, 0:1], t_sb,
                                        start=True, stop=True,
                                    )
                                    prev = (o_ps, l_ps, aidx, (qb % GQA) * CPQ, h)
                                    emit_wo(3, cap=WOPRE)
                            # flush before the collective so its inputs are emitted
                            if prev is not None:
                                emit_norm(prev)
                                prev = None
                            nc.gpsimd.collective_compute(
                                "AllToAll",
                                mybir.AluOpType.bypass,
                                replica_groups=[list(range(NCORES))],
                                ins=[a2a_in[aidx][:, :]],
                                outs=[a2a_out[aidx][:, :]],
                            )
                            emit_at_load(aidx)

                # ---------------- phase 3: row-parallel o_proj on own tokens
                with (
                    tc.tile_pool(name="yout", bufs=4) as y_p,
                    tc.tile_pool(name="psY", bufs=1, space="PSUM") as psy_p,
                ):
                    def at_slice(a, dp):
                        return at_sb[a][dp // DPG][:, dp % DPG]

                    # chunk schedule: (oc, pairgroup). pg=1 (a2,a3) delayed one
                    # oc behind pg=0 so the last AllToAll hides under compute.
                    chunks = []
                    npg = (NA + 1) // 2
                    if npg == 1:
                        chunks = [(oc, 0) for oc in range(OCB)]
                    else:
                        delay = min(2, OCB - 1)
                        for oc in range(delay):
                            chunks.append((oc, 0))
                        for oc in range(delay, OCB):
                            chunks.append((oc, 0))
                            chunks.append((oc - delay, 1))
                        for oc in range(OCB - delay, OCB):
                            chunks.append((oc, 1))

                    for oc, pg in chunks:
                        if pg == 0:
                            emit_wo(DPAIRS)  # ensure this oc's tiles are queued
                        pair = [a for a in (2 * pg, 2 * pg + 1) if a < NA]
                        y_ps = {
                            a: psy_p.tile(
                                [SCA, QB], F32, tag=f"yps{a % 2}", name=f"yps{a}"
                            )
                            for a in pair
                        }
                        for dp in range(DPAIRS):
                            for a in pair:
                                mm_pair(
                                    y_ps[a],
                                    at_slice(a, dp),
                                    wo_sb[(oc, dp)],
                                    dp == 0, dp == DPAIRS - 1, FP8_O,
                                )
                        for ai, a in enumerate(pair):
                            y_sb = y_p.tile([SCA, QB], F32, tag="ysb")
                            if ai % 2 == 0:
                                nc.scalar.mul(y_sb, y_ps[a], float(yscale))
                            else:
                                nc.vector.tensor_scalar_mul(
                                    y_sb, y_ps[a], float(yscale)
                                )
                            nc.scalar.dma_start(
                                out=y_c[
                                    a * SCA : (a + 1) * SCA,
                                    oc * QB : (oc + 1) * QB,
                                ],
                                in_=y_sb,
                            )
    nc.finalize()
    return nc


def _quant8(t, s):
    return np.clip(t * s, -240.0, 240.0).astype(ml_dtypes.float8_e4m3)


def _prep_inputs(hidden_states, wq, wk, wv, wo, position_ids, B, S, H, NH, NKV):
    """Host-side: quantize/cast, x transpose, rope tables, masks, per-core
    weight slices. Returns (in_maps, scales)."""
    BT = B * S
    NHC = NH // NCORES
    DQ = NHC * P

    bf = ml_dtypes.bfloat16
    x2d = np.asarray(hidden_states, dtype=np.float32).reshape(BT, H)
    wq32, wk32, wv32, wo32 = (
        np.asarray(w, dtype=np.float32) for w in (wq, wk, wv, wo)
    )

    if FP8_QKV:
        sx = 240.0 / max(np.abs(x2d).max(), 1e-30)
        swq = 240.0 / max(np.abs(wq32).max(), 1e-30)
        swk = 240.0 / max(np.abs(wk32).max(), 1e-30)
        swv = 240.0 / max(np.abs(wv32).max(), 1e-30)
        xT = np.ascontiguousarray(_quant8(x2d, sx).T)
        wq_b = _quant8(wq32, swq)
        wk_b = _quant8(wk32, swk)
        wv_b = _quant8(wv32, swv)
    else:
        sx = swq = swk = swv = 1.0
        xT = np.ascontiguousarray(x2d.T).astype(bf)
        wq_b, wk_b, wv_b = (w.astype(bf) for w in (wq32, wk32, wv32))

    # |v| upper bound (Cauchy-Schwarz) -> vbig scaled into fp8-safe range
    va = float(
        np.linalg.norm(x2d, axis=1).max() * np.linalg.norm(wv32, axis=0).max()
    )
    cv = VCLIP / va  # |vbig| <= VCLIP in true units
    vscale_dev = cv / (sx * swv)

    if FP8_O:
        swo = 240.0 / max(np.abs(wo32).max(), 1e-30)
        wo_b = _quant8(wo32, swo)
    else:
        swo = 1.0
        wo_b = wo32.astype(bf)
    yscale = 1.0 / (cv * swo)

    escale = (1.0 / math.sqrt(P)) / (sx * sx * swq * swk)

    half = P // 2
    inv_freq = 1.0 / (10000.0 ** (np.arange(half, dtype=np.float64) / half))
    pos = np.asarray(position_ids).astype(np.float64).reshape(BT)
    ang = pos[None, :] * inv_freq[:, None]  # [64, BT]
    cos_t = np.concatenate([np.cos(ang), np.cos(ang)], 0).astype(np.float32)
    sinx_t = np.concatenate([-np.sin(ang), np.sin(ang)], 0).astype(np.float32)

    kk = np.arange(P)[:, None]
    cc = np.arange(P)[None, :]
    tri = (kk <= cc).astype(bf)
    ones_m = np.ones((P, P), dtype=bf)

    in_maps = []
    for c in range(NCORES):
        in_maps.append(
            {
                "xT": xT,
                "wq_c": np.ascontiguousarray(wq_b[:, c * DQ : (c + 1) * DQ]),
                "wk_c": np.ascontiguousarray(wk_b[:, c * P : (c + 1) * P]),
                "wv_c": np.ascontiguousarray(wv_b[:, c * P : (c + 1) * P]),
                "wo_full": wo_b,
                "cos_t": cos_t,
                "sinx_t": sinx_t,
                "tri_t": tri,
                "ones_t": ones_m,
            }
        )
    return in_maps, (escale, vscale_dev, yscale)


def run(hidden_states, wq, wk, wv, wo, position_ids, B, S, H, NH, NKV, trace=False):
    in_maps, scales = _prep_inputs(
        hidden_states, wq, wk, wv, wo, position_ids, B, S, H, NH, NKV
    )
    key = (B, S, H, NH, NKV) + tuple(round(float(s), 14) for s in scales)
    if key not in _CACHE:
        _CACHE[key] = build_program(B, S, H, NH, NKV, *scales)
    nc = _CACHE[key]
    res = run_bass_kernel_spmd(nc, in_maps, core_ids=list(range(NCORES)), trace=trace)
    QBB = S // QB
    GQA = min(2, QBB)
    NQA = (QBB + GQA - 1) // GQA
    NA = B * NQA
    SCA = GQA * QB // NCORES
    out = np.empty((B, S, NH * P), dtype=np.float32)
    for c in range(NCORES):
        yc = res.results[c]["y_c"]
        for a in range(NA):
            b, qq = a // NQA, a % NQA
            tok0 = qq * GQA * QB + c * SCA
            out[b, tok0 : tok0 + SCA, :] = yc[a * SCA : (a + 1) * SCA, :]
    return (out, res) if trace else (out, None)


def kernel(hidden_states, wq, wk, wv, wo, position_ids):
    out, _ = run(
        hidden_states, wq, wk, wv, wo, position_ids, 2, 2048, 4096, 32, 8
    )
    return out
